# revision 2
# baseline (speedup 1.0000x reference)
"""AceStep GQA attention block on 8 TRN2 NeuronCores — v2.

Sharding: TP=2 over head groups x DP=4 over batch (core i: batch i//2,
head group i%2).  Host sums the two Wo partial products per batch.

v2 restructure vs v1: the ACT-engine exp chain (softmax) is the serial
bottleneck (~266us/core), so attention is interleaved with the PE-heavy
projection and output-projection phases instead of running as its own
phase.  Emission schedule:

  S0: proj(pass0)                      PE-bound, psQKV double-buffered
  S1: attn(p0, 8 cps) pumping proj(p1) steps between sk-tiles,
      then attn(p1) cp0-group (cp-major)
  S2: attn(p1) cp1-group pumping oproj(token tiles 0..7)
  S3: oproj(tiles 8..15)

Other changes:
  - Q/K transposes moved off the PE: token-major Q/K tiles are staged to
    DRAM (one DMA per token tile) and read back with dma_start_transpose
    (xbar) straight into QA/KT.
  - softmax denominator: partial sums on DVE+Pool, cross-partition colsum
    via gpsimd.tensor_reduce(axis=C) on Pool, reciprocal on DVE,
    partition_broadcast on Pool — no PE matmuls, no PSUM.
  - psO2 single-buffered; evicted to SBUF (DVE copy) at cp end so the
    next cp's AV chain never waits on the normalization tail.
  - RMSNorm square-sums: ACT in S0 (idle there), DVE tensor_tensor_reduce
    in S1 (ACT is busy with exp); RoPE adds on Pool.
"""

import sys

if "/opt/trn_rl_repo" not in sys.path:
    sys.path.insert(0, "/opt/trn_rl_repo")

from contextlib import ExitStack

import numpy as np
import ml_dtypes

import concourse.bass as bass
import concourse.mybir as mybir
import concourse.tile as tile
import concourse.bass_isa as bass_isa
from concourse.bass import ts, ds
from concourse.vector_clock import ScopedClock, VectorClock
from concourse.bass_utils import run_bass_kernel_spmd

B, S, HID = 4, 2048, 2048
H, KV, D = 16, 4, 128
EPS = 1e-6
SCALE = float(D) ** -0.5
NCORES = 8
TP = 2
QH = H // TP            # 8 q heads per core
KVH = KV // TP          # 2 kv heads per core = passes
QHP = QH // KVH         # 4 q heads per pass
NT = S // 128           # 16 token tiles
NHID = HID // 128       # 16 hid tiles
CH = 512                # oproj chunk width
NCH = S // CH           # 4 chunks
F32 = mybir.dt.float32
F32R = mybir.dt.float32r
BF16 = mybir.dt.bfloat16
CH2 = 1024              # attn sq chunk-pair width (2 PSUM banks)
NCP = S // CH2          # 2 chunk-pairs per head
MULT = mybir.AluOpType.mult
ADD = mybir.AluOpType.add
AF = mybir.ActivationFunctionType
SENTINEL = object()
DEBUG_SINK = [None]


def _patched_drain_and_barrier(self, tick_clock, wait_clock):
    # Walrus CoreV3 rejects >1-2 sem waits on a CTRL (Drain) instruction.
    # Split the final global-clock wait into one single-wait drain per proc.
    gc = tick_clock.global_clock
    n = len(gc)
    for p in range(n):
        t = gc[p]
        if t > 0:
            vec = [0] * n
            vec[p] = t
            d = self.nc.sync.drain()
            wait_clock.add_sem_waits(d.ins, ScopedClock({None: VectorClock(vec)}))
    self.nc.sync.drain()
    self.nc.all_engine_barrier()
    assert self.sems is not None
    popped = self.nc._tile_sem_poison_stack.pop()
    assert popped is self._sem_poison
    self.nc.clear_and_free_semaphores(list(self.sems.allocated().values()))
    self.nc.all_engine_barrier()


tile.TileContext._drain_and_barrier = _patched_drain_and_barrier


def _max_waits(inst):
    if isinstance(inst, mybir.InstEventSemaphore):
        return 2
    return 1


def _legalize_waits(nc):
    """Walrus CoreV3 rejects instructions carrying too many sync waits.
    Spill the excess onto no-op carrier instructions inserted just before,
    on the same engine stream."""
    n_new = 0
    for f in nc.m.functions:
        for bb in f.blocks:
            insts = bb.instructions
            out = []
            changed = False
            for inst in insts:
                si = getattr(inst, "sync_info", None)
                waits = list(si.on_wait) if (si and si.on_wait) else []
                mw = _max_waits(inst)
                if len(waits) > mw:
                    spill, keep = waits[:-mw], waits[-mw:]
                    for i in range(0, len(spill)):
                        nop = mybir.InstNoOp(
                            name=f"waitspill-{n_new}",
                            engine=inst.engine,
                            sync_info=mybir.SyncInfo(
                                on_wait=spill[i : i + 1], on_update=[]
                            ),
                            bass_nofuse=True,
                        )
                        n_new += 1
                        out.append(nop)
                    si.on_wait = keep
                    changed = True
                out.append(inst)
            if changed:
                bb.instructions = out
    return n_new


def _emit(nc, tc, io):
    xT, wq, wkv, rope4, wo, ones_d, ones_bf, qstage, out = io

    xT = xT.rearrange("(j p) t -> p j t", p=128)        # [128, NHID, S]
    wq = wq.rearrange("(j p) n -> p j n", p=128)        # [128, NHID, QH*D]
    wkv = wkv.rearrange("(j p) a n -> p j a n", p=128)  # [128, NHID, KVH, 256]
    wo_r = wo.rearrange("(h p) n -> p h n", p=128)      # [128, QH, HID]

    with ExitStack() as top:
        const = top.enter_context(tc.tile_pool(name="const", bufs=1))
        ones_col = const.tile([128, 1], BF16)
        nc.sync.dma_start(out=ones_col, in_=ones_bf[:, 0:1])
        ones_row = const.tile([1, 128], F32R)
        nc.sync.dma_start(out=ones_row, in_=ones_d[0:1, :])
        eps_t = const.tile([128, 1], F32)
        nc.vector.memset(eps_t, EPS)
        # pin the ACT table set (exp, ln, square, copy) up front
        dummy = const.tile([128, 1], F32)
        nc.scalar.activation(dummy, eps_t, AF.Ln)

        qa_pool = top.enter_context(tc.tile_pool(name="qa", bufs=1))
        QA = qa_pool.tile([128, QH, S], BF16)            # QT, later A, [d, h, t]
        kt_pool = top.enter_context(tc.tile_pool(name="kt", bufs=2))
        v_pool = top.enter_context(tc.tile_pool(name="v", bufs=2))
        KTs, VTs = [], []
        for p in range(KVH):
            KTs.append(kt_pool.tile([128, S], BF16, tag="kt", name=f"kt{p}"))
            VTs.append(v_pool.tile([128, NT, D], BF16, tag="vt", name=f"vt{p}"))

        epool = top.enter_context(tc.tile_pool(name="e", bufs=6))
        wq_pool = top.enter_context(tc.tile_pool(name="wq", bufs=1))
        small = top.enter_context(tc.tile_pool(name="small", bufs=4))
        spool = top.enter_context(tc.tile_pool(name="scr", bufs=6))
        xpool = top.enter_context(tc.tile_pool(name="x", bufs=3))
        rpool = top.enter_context(tc.tile_pool(name="rope", bufs=3))
        qkpool = top.enter_context(tc.tile_pool(name="qk", bufs=3))
        accpool = top.enter_context(tc.tile_pool(name="acc", bufs=1))
        osbpool = top.enter_context(tc.tile_pool(name="osb", bufs=2))
        bcpool = top.enter_context(tc.tile_pool(name="bc", bufs=1))
        dnpool = top.enter_context(tc.tile_pool(name="dn", bufs=2))
        sbpool = top.enter_context(tc.tile_pool(name="sb", bufs=2))

        def load_weights(p):
            wq_sb = wq_pool.tile([128, NHID, QHP * D], BF16, tag="wq", name="wq_sb")
            wkv_sb = wq_pool.tile([128, NHID, 256], BF16, tag="wkv", name="wkv_sb")
            for jq in range(4):
                nc.sync.dma_start(
                    out=wq_sb[:, ds(jq * 4, 4), :],
                    in_=wq[:, ds(jq * 4, 4), ds(p * QHP * D, QHP * D)],
                )
            nc.sync.dma_start(out=wkv_sb, in_=wkv[:, :, p, :])
            return wq_sb, wkv_sb

        cur_w = [load_weights(0), None]

        # ---------------- projection pass emitter (generator) ----------------
        # PSUM per tile: two 1-bank half-tiles from pjpool:
        #   A = [Q heads {0,1} (256) | K (128) | V (128)]
        #   B = [Q heads {2,3} (256) | unused]
        # Half-tile granularity doubles the effective buffering per bank, so
        # tile t+1's matmuls only wait on tile t's half-tile read chains.
        def proj_steps(p, pjpool):
            wq_sb, wkv_sb = cur_w[p]
            KT_p, VT_p = KTs[p], VTs[p]
            for tt in range(NT):
                xx = xpool.tile([128, NHID, 128], BF16, tag="xx", name="xx")
                nc.scalar.dma_start(out=xx, in_=xT[:, :, ts(tt, 128)])
                rp = rpool.tile([128, 4, 128], F32, tag="rp", name="rp")
                nc.sync.dma_start(out=rp, in_=rope4[ts(tt, 128), :, :])
                cwq_t = rp[:, 0, :]
                swq_t = rp[:, 1, :]
                cwk_t = rp[:, 2, :]
                swk_t = rp[:, 3, :]
                yield

                psQ = pjpool.tile([128, 512], F32, tag="pj", name="psQ")
                for j0 in range(0, NHID, 2):
                    for j in (j0, j0 + 1):
                        nc.tensor.matmul(
                            psQ,
                            xx[:, j, :],
                            wq_sb[:, j, :],
                            start=(j == 0),
                            stop=(j == NHID - 1),
                        )
                    yield
                # evict Q to SBUF immediately so the PSUM bank recycles after
                # one DVE op; the norm/rope chain reads the SBUF copy
                sbA = sbpool.tile([128, 512], F32, tag="sbA", name="sbA")
                nc.scalar.copy(sbA, psQ)
                yield
                psKV = pjpool.tile([128, 512], F32, tag="pj", name="psKV")
                for j0 in range(0, NHID, 4):
                    for j in range(j0, j0 + 4):
                        nc.tensor.matmul(
                            psKV[:, 0:256],
                            xx[:, j, :],
                            wkv_sb[:, j, :],
                            start=(j == 0),
                            stop=(j == NHID - 1),
                        )
                    yield
                sbB = sbpool.tile([128, 256], F32, tag="sbB", name="sbB")
                nc.scalar.copy(sbB, psKV[:, 0:256])
                yield

                # RMSNorm scales: 5 square-sums (4 Q heads + K) -> one batch
                ssq5 = small.tile([128, 5], F32, tag="ssq", name="ssq")
                r5 = small.tile([128, 5], F32, tag="r", name="r5")
                scratch = spool.tile([128, 128], F32, tag="scr", name="scr")
                for jh in range(QHP):
                    nc.scalar.activation(
                        scratch,
                        sbA[:, ts(jh, 128)],
                        AF.Square,
                        accum_out=ssq5[:, jh : jh + 1],
                    )
                nc.scalar.activation(
                    scratch, sbB[:, 0:128], AF.Square, accum_out=ssq5[:, 4:5]
                )
                s15 = small.tile([128, 5], F32, tag="s1", name="s15")
                nc.scalar.activation(s15, ssq5, AF.Ln, bias=eps_t, scale=1.0 / D)
                nc.scalar.activation(r5, s15, AF.Exp, scale=-0.5)
                yield

                qk = qkpool.tile([128, QHP + 1, 128], BF16, tag="qk", name="qk")

                def norm_rope(src, cw_t, sw_t, r, dst):
                    m1 = spool.tile([128, 128], F32, tag="m1", name="m1")
                    m2 = spool.tile([128, 128], F32, tag="m2", name="m2")
                    nc.vector.scalar_tensor_tensor(
                        out=m1, in0=src, scalar=r, in1=cw_t, op0=MULT, op1=MULT
                    )
                    nc.vector.scalar_tensor_tensor(
                        out=m2[:, 0:64],
                        in0=src[:, 64:128],
                        scalar=r,
                        in1=sw_t[:, 0:64],
                        op0=MULT,
                        op1=MULT,
                    )
                    nc.vector.scalar_tensor_tensor(
                        out=m2[:, 64:128],
                        in0=src[:, 0:64],
                        scalar=r,
                        in1=sw_t[:, 64:128],
                        op0=MULT,
                        op1=MULT,
                    )
                    nc.gpsimd.tensor_add(dst, m1, m2)

                for jh in range(QHP):
                    norm_rope(
                        sbA[:, ts(jh, 128)], cwq_t, swq_t, r5[:, jh : jh + 1],
                        qk[:, jh, :],
                    )
                    yield
                norm_rope(
                    sbB[:, 0:128], cwk_t, swk_t, r5[:, 4:5], qk[:, QHP, :]
                )
                nc.scalar.copy(VT_p[:, tt, :], sbB[:, 128:256])
                yield
                nc.sync.dma_start(
                    out=qstage[ds(p, 1), ts(tt, 128), :, :], in_=qk
                )
                yield
                if tt == NT // 2 - 1:
                    # tiles 0..7 staged: transpose the cp0 halves of the
                    # first q head and K so the next attn group can start
                    # the moment the pass's last tile lands
                    h0 = p * QHP
                    nc.sync.dma_start_transpose(
                        out=QA[:, h0, 0:CH2], in_=qstage[p, 0:CH2, 0, :]
                    )
                    nc.sync.dma_start_transpose(
                        out=KT_p[:, 0:CH2], in_=qstage[p, 0:CH2, QHP, :]
                    )
                    yield
            # remaining transpose-reads, split across both hwdge queues in
            # consumption order (cp0 slices of later heads before cp1 tails)
            h0 = p * QHP
            nc.sync.dma_start_transpose(
                out=QA[:, h0, CH2:S], in_=qstage[p, CH2:S, 0, :]
            )
            nc.sync.dma_start_transpose(
                out=KT_p[:, CH2:S], in_=qstage[p, CH2:S, QHP, :]
            )
            yield
            for jh in range(1, QHP):
                eng = nc.sync
                eng.dma_start_transpose(
                    out=QA[:, h0 + jh, 0:CH2], in_=qstage[p, 0:CH2, jh, :]
                )
                eng.dma_start_transpose(
                    out=QA[:, h0 + jh, CH2:S], in_=qstage[p, CH2:S, jh, :]
                )
                yield


        def drain(gen):
            for _ in gen:
                pass

        def make_pump(gen):
            def pump(n):
                for _ in range(n):
                    if next(gen, SENTINEL) is SENTINEL:
                        return
            return pump

        def pump_noop(n):
            pass

        # ---------------- attention chunk-pair emitter ----------------
        # pss/pso are created after S0 (late-bound in attn_cp) so their PSUM
        # banks don't coexist with pj0's double buffers.
        pss = pso = None
        pending_tail = [None]
        pump_n = [4]

        def attn_cp(p, hl, cp, pump):
            KT_p, VT_p = KTs[p], VTs[p]
            etiles = [None] * NT

            def scores(i):
                psS = pss.tile([128, CH2], F32, tag="s", name="psS")
                for h2 in range(2):
                    nc.tensor.matmul(
                        psS[:, ds(h2 * CH, CH)],
                        KT_p[:, ts(i, 128)],
                        QA[:, hl, ds(cp * CH2 + h2 * CH, CH)],
                        start=True,
                        stop=True,
                    )
                e = epool.tile([128, CH2], BF16, tag="e", name="e")
                nc.scalar.activation(e, psS, AF.Exp, scale=SCALE)
                etiles[i] = e

            psO2 = pso.tile([128, CH2], F32, tag="o", name="psO2")
            scores(0)
            scores(1)
            accs = None
            for i in range(NT):
                if i == 2:
                    if pending_tail[0] is not None:
                        pending_tail[0]()
                        pending_tail[0] = None
                    # allocated after the previous tail so the pool WAR deps
                    # resolve in emission order (the tail reads the old accs)
                    accs = [
                        accpool.tile(
                            [128, CH2], BF16, tag=f"acc{g}", name=f"acc{g}"
                        )
                        for g in range(4)
                    ]
                e = etiles[i]
                for h2 in range(2):
                    nc.tensor.matmul(
                        psO2[:, ds(h2 * CH, CH)],
                        VT_p[:, i, :],
                        e[:, ds(h2 * CH, CH)],
                        start=(i == 0),
                        stop=(i == NT - 1),
                    )
                if i + 2 < NT:
                    scores(i + 2)
                # softmax denominator partial sums: 4 stride-4 chains on DVE
                # (bf16 2x mode, 594ns each, spread across the cp)
                g = i % 4
                eng = nc.gpsimd if g == 3 else nc.vector
                if 4 <= i < 8:
                    eng.tensor_add(accs[g], etiles[i - 4], e)
                elif i >= 8:
                    eng.tensor_add(accs[g], accs[g], e)
                # pump only after the deferred tail has been emitted: pumped
                # oproj units read QA slices the tail writes, and tile deps
                # resolve in emission order
                if i >= 3:
                    pump(pump_n[0])

            # cp end: combine partial sums in place, evict psO2 to SBUF.
            # a23 on Pool (it ends the Pool chain anyway), eviction on ACT —
            # keeps the DVE clump at two adds so proj evictions don't stall.
            nc.vector.tensor_add(accs[0], accs[0], accs[1])
            nc.vector.tensor_add(accs[2], accs[2], accs[3])
            nc.vector.tensor_add(accs[0], accs[0], accs[2])
            ACCt = accs[0]
            O_sb = osbpool.tile([128, CH2], F32, tag="o_sb", name="O_sb")
            nc.vector.tensor_copy(O_sb, psO2)

            def tail():
                psD = pss.tile([1, CH2], F32, tag="s", name="psD")
                nc.tensor.matmul(
                    psD[:, 0:CH], ones_col, ACCt[:, 0:CH], start=True, stop=True
                )
                nc.tensor.matmul(
                    psD[:, CH:CH2], ones_col, ACCt[:, CH:CH2],
                    start=True, stop=True,
                )
                rd = dnpool.tile([1, CH2], F32R, tag="rd", name="rd")
                with nc.allow_low_precision(reason="f32r bcast rhs"):
                    nc.vector.reciprocal(rd, psD)
                psB = pss.tile([128, CH2], F32, tag="s", name="psB")
                nc.tensor.matmul(
                    psB[:, 0:CH], ones_row, rd[:, 0:CH], start=True, stop=True
                )
                nc.tensor.matmul(
                    psB[:, CH:CH2], ones_row, rd[:, CH:CH2],
                    start=True, stop=True,
                )
                nc.vector.tensor_mul(
                    QA[:, hl, ds(cp * CH2, CH2)], O_sb, psB
                )

            pending_tail[0] = tail

        # ---------------- oproj emitter (generator) ----------------
        wo_holder = [None]

        def oproj_steps(tts):
            wo_sb = wo_holder[0]
            for k, tt in enumerate(tts):
                for half in range(2):
                    osb = osbpool.tile([128, 2, CH], F32, tag="osb", name="osb")
                    for nc2 in range(2):
                        nch = half * 2 + nc2
                        psC = psc_holder[0].tile(
                            [128, CH], F32, tag="c", name="psC"
                        )
                        for h in range(QH):
                            nc.tensor.matmul(
                                psC,
                                QA[:, h, ts(tt, 128)],
                                wo_sb[:, h, ds(nch * CH, CH)],
                                start=(h == 0),
                                stop=(h == QH - 1),
                            )
                            if h % 4 == 3:
                                yield
                        if nch % 2 == 0:
                            nc.vector.tensor_copy(osb[:, nc2, :], psC)
                        else:
                            nc.scalar.copy(osb[:, nc2, :], psC)
                        yield
                    nc.sync.dma_start(
                        out=out[ts(tt, 128), ds(half * 2 * CH, 2 * CH)].rearrange(
                            "p (a c) -> p a c", a=2
                        ),
                        in_=osb,
                    )
                    yield

        psc_holder = [None]

        # ================= schedule =================
        # S0: projection pass 0, standalone (double-buffered PSUM)
        with tc.tile_pool(name="pj0", bufs=6, space="PSUM") as pj0:
            drain(proj_steps(0, pj0))

        # S1: attn(p0) pumping proj(p1); then attn(p1) cp0-group
        pss = top.enter_context(tc.tile_pool(name="pss", bufs=2, space="PSUM"))
        pso = top.enter_context(tc.tile_pool(name="pso", bufs=1, space="PSUM"))
        with tc.tile_pool(name="pj1", bufs=2, space="PSUM") as pj1:
            def gen1():
                cur_w[1] = load_weights(1)
                yield
                yield from proj_steps(1, pj1)

            g1 = make_pump(gen1())
            for jh in range(QHP):
                for cp in range(NCP):
                    attn_cp(0, jh, cp, g1)
            g1(10 ** 6)
            # load Wo while p1-cp0 group runs
            wo_pool = top.enter_context(tc.tile_pool(name="wo", bufs=1))
            wo_sb = wo_pool.tile([128, QH, HID], BF16, name="wo_sb")
            nc.sync.dma_start(out=wo_sb, in_=wo_r)
            wo_holder[0] = wo_sb
            for jh in range(QHP):
                attn_cp(1, QHP + jh, 0, pump_noop)

        # S2: attn(p1) cp1-group pumping oproj(tiles 0..7); S3: rest
        with tc.tile_pool(name="psc", bufs=2, space="PSUM") as psc:
            psc_holder[0] = psc
            pump_n[0] = 2
            g2 = make_pump(oproj_steps(range(0, 8)))
            for jh in range(QHP):
                attn_cp(1, QHP + jh, 1, g2)
            if pending_tail[0] is not None:
                pending_tail[0]()
                pending_tail[0] = None
            g2(10 ** 6)
            drain(oproj_steps(range(8, NT)))
        if DEBUG_SINK[0] is not None:
            for jh in range(QH):
                nc.sync.dma_start(
                    out=DEBUG_SINK[0][:, jh, :], in_=QA[:, jh, :]
                )


_PROGRAM = None


def _build_program(legalize=True, bodies=1):
    global _PROGRAM
    if _PROGRAM is not None and legalize and bodies == 1:
        return _PROGRAM
    nc = bass.Bass("TRN2", target_bir_lowering=False, debug=False, num_devices=NCORES)
    xT = nc.dram_tensor("xT", [HID, S], BF16, kind="ExternalInput").ap()
    wq = nc.dram_tensor("wq", [HID, QH * D], BF16, kind="ExternalInput").ap()
    wkv = nc.dram_tensor("wkv", [HID, KVH, 256], BF16, kind="ExternalInput").ap()
    rope4 = nc.dram_tensor("rope4", [S, 4, D], F32, kind="ExternalInput").ap()
    wo = nc.dram_tensor("wo", [QH * D, HID], BF16, kind="ExternalInput").ap()
    ones_d = nc.dram_tensor("ones", [128, 128], F32R, kind="ExternalInput").ap()
    ones_bf = nc.dram_tensor("ones_bf", [128, 2], BF16, kind="ExternalInput").ap()
    out = nc.dram_tensor("out", [S, HID], F32, kind="ExternalOutput").ap()
    with tile.TileContext(nc) as tc:
        for rep in range(bodies):
            qstage = nc.dram_tensor(
                f"qstage{rep}", [KVH, S, QHP + 1, D], BF16, kind="Internal"
            ).ap()
            _emit(nc, tc, (xT, wq, wkv, rope4, wo, ones_d, ones_bf, qstage, out))
    if legalize:
        _legalize_waits(nc)
        if bodies == 1:
            _PROGRAM = nc
    return nc


def _host_prep(hidden_states, cos, sin, Wq, Wk, Wv, Wo, q_norm_w, k_norm_w):
    """Build per-core input maps."""
    f = np.float32
    cos = np.asarray(cos, f)
    sin = np.asarray(sin, f)
    qw = np.asarray(q_norm_w, f)
    kw = np.asarray(k_norm_w, f)

    def fold(w):
        cw = (cos * w[None, :]).astype(f)
        sw = np.empty_like(sin)
        half = D // 2
        sw[:, :half] = -sin[:, :half] * w[None, half:]
        sw[:, half:] = sin[:, half:] * w[None, :half]
        return np.ascontiguousarray(cw), np.ascontiguousarray(sw)

    cwq, swq = fold(qw)
    cwk, swk = fold(kw)
    rope4 = np.stack([cwq, swq, cwk, swk], axis=1)  # [S, 4, D]

    Wq = np.asarray(Wq, f)
    Wk = np.asarray(Wk, f)
    Wv = np.asarray(Wv, f)
    Wo = np.asarray(Wo, f)
    hs = np.asarray(hidden_states, f)

    bf = ml_dtypes.bfloat16
    in_maps = []
    for i in range(NCORES):
        b, g = i // TP, i % TP
        xT = np.ascontiguousarray(hs[b].T).astype(bf)           # [HID, S]
        wq_g = np.ascontiguousarray(Wq[:, g * QH * D:(g + 1) * QH * D]).astype(bf)
        wkv = np.empty((HID, KVH, 256), f)
        for p in range(KVH):
            kvh = g * KVH + p
            wkv[:, p, 0:128] = Wk[:, kvh * D:(kvh + 1) * D]
            wkv[:, p, 128:256] = Wv[:, kvh * D:(kvh + 1) * D]
        wkv = wkv.astype(bf)
        wo_g = np.ascontiguousarray(Wo[g * QH * D:(g + 1) * QH * D, :]).astype(bf)
        in_maps.append(
            {
                "xT": xT,
                "wq": wq_g,
                "wkv": wkv,
                "rope4": rope4,
                "wo": wo_g,
                "ones": np.ones((128, 128), f),
                "ones_bf": np.ones((128, 2), ml_dtypes.bfloat16),
            }
        )
    return in_maps


def run_cores(in_maps, trace=False, **kwargs):
    nc = _build_program()
    return run_bass_kernel_spmd(
        nc, in_maps, core_ids=list(range(NCORES)), trace=trace, **kwargs
    )


def kernel(hidden_states, cos, sin, Wq, Wk, Wv, Wo, q_norm_w, k_norm_w):
    in_maps = _host_prep(
        hidden_states, cos, sin, Wq, Wk, Wv, Wo, q_norm_w, k_norm_w
    )
    res = run_cores(in_maps, trace=False)
    out = np.empty((B, S, HID), np.float32)
    for b in range(B):
        out[b] = res.results[b * TP]["out"]
        out[b] += res.results[b * TP + 1]["out"]
    return out


# revision 4
# speedup vs baseline: 1.0215x; 1.0215x over previous
"""AceStep GQA attention block on 8 TRN2 NeuronCores.

Sharding: tensor-parallel over heads (TP=2, kv heads stay grouped with
their q heads) x data-parallel over batch (DP=4).  Core i handles batch
b = i // 2 and head group g = i % 2 (q heads 8g..8g+7, kv heads 2g,2g+1).
Each core computes a partial output projection (its head group's slice of
Wo rows); the host sums the two partials per batch.

Device-side dataflow per core (all matmuls f32r = full-rate fp32):
  pass p in {0,1}:  (kv head p, q heads 4p..4p+3)
    proj:  xT tiles (stationary) x Wq/Wk/Wv slices -> Q/K/V token-major,
           per-head RMSNorm via ACT square+accum, rsqrt; RoPE fused with the
           norm scale via scalar_tensor_tensor (cos/sin tables carry the
           norm weights, folded on host); PE-transpose Q,K to [d, t].
    attn:  ST[sk,sq] = KT_tile.T @ QT chunk; E = exp(SCALE*ST) on ACT;
           denominator = ones.T @ E (PE, accumulated over sk tiles);
           OUT_T[d,sq] = V_tile.T @ E accumulated over sk tiles;
           A = OUT_T * bcast(1/denom)  (bcast via K=1 matmul).
  final: out[t,:] += A_h[:,t].T @ Wo_h rows, accumulated over 8 heads.
"""

import sys

if "/opt/trn_rl_repo" not in sys.path:
    sys.path.insert(0, "/opt/trn_rl_repo")

from contextlib import ExitStack

import numpy as np
import ml_dtypes

import concourse.bass as bass
import concourse.mybir as mybir
import concourse.tile as tile
from concourse.bass import ts, ds
from concourse.masks import make_identity
from concourse.vector_clock import ScopedClock, VectorClock
from concourse.bass_utils import run_bass_kernel_spmd

B, S, HID = 4, 2048, 2048
H, KV, D = 16, 4, 128
EPS = 1e-6
SCALE = float(D) ** -0.5
NCORES = 8
TP = 2
QH = H // TP            # 8 q heads per core
KVH = KV // TP          # 2 kv heads per core = passes
QHP = QH // KVH         # 4 q heads per pass
NT = S // 128           # 16 token tiles
NHID = HID // 128       # 16 hid tiles
CH = 512                # sq chunk width
NCH = S // CH           # 4 chunks
F32 = mybir.dt.float32
F32R = mybir.dt.float32r
BF16 = mybir.dt.bfloat16
CH2 = 1024              # paired sq chunk width (2 PSUM banks)
MULT = mybir.AluOpType.mult
AF = mybir.ActivationFunctionType


def _patched_drain_and_barrier(self, tick_clock, wait_clock):
    # Walrus CoreV3 rejects >1-2 sem waits on a CTRL (Drain) instruction.
    # Split the final global-clock wait into one single-wait drain per proc.
    gc = tick_clock.global_clock
    n = len(gc)
    for p in range(n):
        t = gc[p]
        if t > 0:
            vec = [0] * n
            vec[p] = t
            d = self.nc.sync.drain()
            wait_clock.add_sem_waits(d.ins, ScopedClock({None: VectorClock(vec)}))
    self.nc.sync.drain()
    self.nc.all_engine_barrier()
    assert self.sems is not None
    popped = self.nc._tile_sem_poison_stack.pop()
    assert popped is self._sem_poison
    self.nc.clear_and_free_semaphores(list(self.sems.allocated().values()))
    self.nc.all_engine_barrier()


tile.TileContext._drain_and_barrier = _patched_drain_and_barrier

def _max_waits(inst):
    # Walrus CoreV2/V3 setupSyncWait takes a single wait per TPB instruction;
    # EventSemaphore can hold two.
    if isinstance(inst, mybir.InstEventSemaphore):
        return 2
    return 1


def _legalize_waits(nc):
    """Walrus CoreV3 rejects instructions carrying too many sync waits.
    Spill the excess onto no-op carrier instructions inserted just before,
    on the same engine stream."""
    n_new = 0
    for f in nc.m.functions:
        for bb in f.blocks:
            insts = bb.instructions
            out = []
            changed = False
            for inst in insts:
                si = getattr(inst, "sync_info", None)
                waits = list(si.on_wait) if (si and si.on_wait) else []
                mw = _max_waits(inst)
                if len(waits) > mw:
                    spill, keep = waits[:-mw], waits[-mw:]
                    for i in range(0, len(spill)):
                        nop = mybir.InstNoOp(
                            name=f"waitspill-{n_new}",
                            engine=inst.engine,
                            sync_info=mybir.SyncInfo(
                                on_wait=spill[i : i + 1], on_update=[]
                            ),
                            bass_nofuse=True,
                        )
                        n_new += 1
                        out.append(nop)
                    si.on_wait = keep
                    changed = True
                out.append(inst)
            if changed:
                bb.instructions = out
    return n_new


def _emit(nc, tc, io, phases=("proj", "attn", "oproj")):
    xT, wq, wkv, rope4, wo, ones_d, ones_bf, out = io

    xT = xT.rearrange("(j p) t -> p j t", p=128)       # [128, NHID, S]
    wq = wq.rearrange("(j p) n -> p j n", p=128)       # [128, NHID, QH*D]
    wkv = wkv.rearrange("(j p) a n -> p j a n", p=128)  # [128, NHID, KVH, 256]
    wo_r = wo.rearrange("(h p) n -> p h n", p=128)     # [128, QH, HID]

    with ExitStack() as top:
        const = top.enter_context(tc.tile_pool(name="const", bufs=1))
        ident = const.tile([128, 128], F32)
        make_identity(nc, ident)
        ones_col = const.tile([128, 1], BF16)
        nc.sync.dma_start(out=ones_col, in_=ones_bf[:, 0:1])
        ones_row = const.tile([1, 128], F32R)
        nc.sync.dma_start(out=ones_row, in_=ones_d[0:1, :])
        eps_t = const.tile([128, 1], F32)
        nc.vector.memset(eps_t, EPS)
        # pin the ACT table set to natural_log_exp_and_others (has exp, ln,
        # square, copy) so no table switches happen mid-kernel
        dummy = const.tile([128, 1], F32)
        nc.scalar.activation(dummy, eps_t, AF.Ln)

        qa_pool = top.enter_context(tc.tile_pool(name="qa", bufs=1))
        QA = qa_pool.tile([128, QH, S], BF16)           # QT, later A, [d, h, t]
        kt_pool = top.enter_context(tc.tile_pool(name="kt", bufs=1))
        KT = kt_pool.tile([128, S], BF16)               # per-pass KT [d, t]
        v_pool = top.enter_context(tc.tile_pool(name="v", bufs=1))
        VT = v_pool.tile([128, NT, D], BF16)            # per-pass V [t%128, tt, d]

        epool = top.enter_context(tc.tile_pool(name="e", bufs=10))
        wq_pool = top.enter_context(tc.tile_pool(name="wq", bufs=2))

        def load_weights(p):
            wq_sb = wq_pool.tile([128, NHID, QHP * D], BF16, tag="wq", name="wq_sb")
            wkv_sb = wq_pool.tile([128, NHID, 256], BF16, tag="wkv", name="wkv_sb")
            for jq in range(4):
                nc.scalar.dma_start(
                    out=wq_sb[:, ds(jq * 4, 4), :],
                    in_=wq[:, ds(jq * 4, 4), ds(p * QHP * D, QHP * D)],
                )
            nc.scalar.dma_start(out=wkv_sb, in_=wkv[:, :, p, :])
            return wq_sb, wkv_sb

        cur_w = load_weights(0) if "proj" in phases else None
        small = top.enter_context(tc.tile_pool(name="small", bufs=4))

        wo_sb = None
        oproj_pools = [None, None]  # (psc, opool)

        def oproj_gen(tts):
            psc_p, opool_p = oproj_pools
            for tt in tts:
                for half in range(2):
                    osb = opool_p.tile([128, 2, CH], F32, tag="osb", name="osb")
                    for nc2 in range(2):
                        nch = half * 2 + nc2
                        psC = psc_p.tile([128, CH], F32, tag="c", name="psC")
                        for h in range(QH):
                            nc.tensor.matmul(
                                psC,
                                QA[:, h, ts(tt, 128)],
                                wo_sb[:, h, ds(nch * CH, CH)],
                                start=(h == 0),
                                stop=(h == QH - 1),
                            )
                            if h % 4 == 3:
                                yield True
                        if nch % 2 == 0:
                            nc.scalar.copy(osb[:, nc2, :], psC)
                        else:
                            nc.vector.tensor_copy(osb[:, nc2, :], psC)
                        yield True
                    nc.sync.dma_start(
                        out=out[ts(tt, 128), ds(half * 2 * CH, 2 * CH)].rearrange(
                            "p (a c) -> p a c", a=2
                        ),
                        in_=osb,
                    )
                    yield True

        for p in range(KVH):
            # ---------------- projection phase (pass p) ----------------
            with ExitStack() as ph:
                if "proj" in phases:
                    wq_sb, wkv_sb = cur_w
                    xpool = ph.enter_context(tc.tile_pool(name="x", bufs=6))
                    rpool = ph.enter_context(tc.tile_pool(name="rope", bufs=3))
                    spool = ph.enter_context(tc.tile_pool(name="scr", bufs=4))
                    qrpool = ph.enter_context(tc.tile_pool(name="qr", bufs=10))
                    psq = ph.enter_context(tc.tile_pool(name="psq", bufs=3, space="PSUM"))
                    pskv = ph.enter_context(tc.tile_pool(name="pskv", bufs=2, space="PSUM"))
                    pst_pool = ph.enter_context(
                        tc.tile_pool(name="pst", bufs=3, space="PSUM")
                    )

                    # transpose+copy of tile tt is deferred until after tile
                    # tt+1's projection matmuls so the PE never waits on the
                    # ACT/DVE norm+rope chain.
                    pending = []
                    new_pending = []

                    def flush_pending():
                        for qr_t, dst in pending:
                            psT = pst_pool.tile([128, 128], F32)
                            nc.tensor.transpose(psT, qr_t, ident)
                            nc.scalar.copy(dst, psT)
                        pending.clear()

                    for tt in range(NT):
                        xx = xpool.tile([128, NHID, 128], BF16, tag="xx")
                        nc.sync.dma_start(out=xx, in_=xT[:, :, ts(tt, 128)])
                        rp = rpool.tile([128, 4, 128], F32, tag="rp")
                        nc.sync.dma_start(out=rp, in_=rope4[ts(tt, 128), :, :])
                        cwq_t = rp[:, 0, :]
                        swq_t = rp[:, 1, :]
                        cwk_t = rp[:, 2, :]
                        swk_t = rp[:, 3, :]

                        psQ = psq.tile([128, QHP * D], F32)
                        psKV = pskv.tile([128, 256], F32)
                        for j in range(NHID):
                            nc.tensor.matmul(
                                psQ,
                                xx[:, j, :],
                                wq_sb[:, j, :],
                                start=(j == 0),
                                stop=(j == NHID - 1),
                            )
                        for j in range(NHID):
                            nc.tensor.matmul(
                                psKV,
                                xx[:, j, :],
                                wkv_sb[:, j, :],
                                start=(j == 0),
                                stop=(j == NHID - 1),
                            )

                        # batched RMSNorm scale: 5 squares (4 Q heads + K)
                        # accumulate into one [128,5]; one ln + one exp.
                        scratch = spool.tile([128, 128], F32, tag="scr")
                        ssq5 = small.tile([128, 5], F32, tag="ssq")
                        s15 = small.tile([128, 5], F32, tag="s1")
                        r5 = small.tile([128, 5], F32, tag="r")
                        for jh in range(QHP):
                            nc.scalar.activation(
                                scratch,
                                psQ[:, ts(jh, 128)],
                                AF.Square,
                                accum_out=ssq5[:, jh : jh + 1],
                            )
                        nc.scalar.activation(
                            scratch,
                            psKV[:, 0:128],
                            AF.Square,
                            accum_out=ssq5[:, 4:5],
                        )
                        nc.scalar.activation(s15, ssq5, AF.Ln, bias=eps_t, scale=1.0 / D)
                        nc.scalar.activation(r5, s15, AF.Exp, scale=-0.5)

                        def norm_rope(src, cw_t, sw_t, r, dst):
                            m1 = spool.tile([128, 128], F32, tag="m1")
                            m2 = spool.tile([128, 128], F32, tag="m2")
                            qr = qrpool.tile([128, 128], F32, tag="qr")
                            nc.vector.scalar_tensor_tensor(
                                out=m1, in0=src, scalar=r, in1=cw_t, op0=MULT, op1=MULT
                            )
                            nc.vector.scalar_tensor_tensor(
                                out=m2[:, 0:64],
                                in0=src[:, 64:128],
                                scalar=r,
                                in1=sw_t[:, 0:64],
                                op0=MULT,
                                op1=MULT,
                            )
                            nc.vector.scalar_tensor_tensor(
                                out=m2[:, 64:128],
                                in0=src[:, 0:64],
                                scalar=r,
                                in1=sw_t[:, 64:128],
                                op0=MULT,
                                op1=MULT,
                            )
                            nc.vector.tensor_add(qr, m1, m2)
                            new_pending.append((qr, dst))

                        for jh in range(QHP):
                            hl = p * QHP + jh
                            norm_rope(
                                psQ[:, ts(jh, 128)],
                                cwq_t,
                                swq_t,
                                r5[:, jh : jh + 1],
                                QA[:, hl, ts(tt, 128)],
                            )
                        norm_rope(
                            psKV[:, 0:128],
                            cwk_t,
                            swk_t,
                            r5[:, 4:5],
                            KT[:, ts(tt, 128)],
                        )
                        nc.scalar.copy(VT[:, tt, :], psKV[:, 128:256])
                        flush_pending()
                        pending.extend(new_pending)
                        new_pending.clear()
                    flush_pending()
                    if p + 1 < KVH:
                        cur_w = load_weights(p + 1)

            # load Wo after the last projection phase frees its pools
            if p == KVH - 1 and "oproj" in phases:
                wo_pool = top.enter_context(tc.tile_pool(name="wo", bufs=1))
                wo_sb = wo_pool.tile([128, QH, HID], BF16)
                nc.sync.dma_start(out=wo_sb, in_=wo_r)
                oproj_pools[0] = top.enter_context(
                    tc.tile_pool(name="psc", bufs=2, space="PSUM")
                )
                oproj_pools[1] = top.enter_context(
                    tc.tile_pool(name="osb2", bufs=3)
                )

            # ---------------- attention phase (pass p) ----------------
            if "attn" not in phases:
                continue
            # Processed in sq chunk-pairs of 1024: scores fill a 2-bank PSUM
            # tile, one wide exp per sk tile (amortizes ACT per-op overhead),
            # denominator 2-way column-tiled on the PE (concurrent groups).
            with ExitStack() as ph:
                accpool = ph.enter_context(tc.tile_pool(name="acc", bufs=1))
                osbp = ph.enter_context(tc.tile_pool(name="osbp", bufs=2))
                pss = ph.enter_context(tc.tile_pool(name="pss", bufs=2, space="PSUM"))
                pso = ph.enter_context(tc.tile_pool(name="pso", bufs=1, space="PSUM"))

                # The normalization tail of chunk-pair K (denominator
                # colsum, reciprocal, broadcast, final multiply) is deferred
                # into chunk-pair K+1's stream so the PE never waits on the
                # DVE/Pool partial-sum chains.
                tail_prev = [None]

                def make_tail(hl, cp, O_sb, accs):
                    def tail():
                        psD = [
                            pss.tile([1, CH], F32, tag="s", name=f"psD{_h}")
                            for _h in range(2)
                        ]
                        for h2 in range(2):
                            for gi in range(4):
                                nc.tensor.matmul(
                                    psD[h2],
                                    ones_col,
                                    accs[gi][:, ds(h2 * CH, CH)],
                                    start=(gi == 0),
                                    stop=(gi == 3),
                                )
                        for h2 in range(2):
                            c = cp * 2 + h2
                            rd = small.tile([1, CH], F32R, tag="rd")
                            with nc.allow_low_precision(reason="f32r bcast rhs"):
                                nc.vector.reciprocal(rd, psD[h2])
                            psB = pss.tile([128, CH], F32, tag="s")
                            nc.tensor.matmul(
                                psB, ones_row, rd, start=True, stop=True
                            )
                            bc = epool.tile([128, CH], F32, tag="bc")
                            nc.vector.tensor_copy(bc, psB)
                            nc.vector.tensor_mul(
                                QA[:, hl, ds(c * CH, CH)],
                                O_sb[:, ds(h2 * CH, CH)],
                                bc,
                            )
                    return tail

                def attn_cp(hl, cp, pump):
                    etiles = [None] * NT

                    def scores(i):
                        psS = pss.tile([128, CH2], F32, tag="s")
                        for h2 in range(2):
                            nc.tensor.matmul(
                                psS[:, ds(h2 * CH, CH)],
                                KT[:, ts(i, 128)],
                                QA[:, hl, ds(cp * CH2 + h2 * CH, CH)],
                                start=True,
                                stop=True,
                            )
                        e = epool.tile([128, CH2], BF16, tag="e")
                        nc.scalar.activation(e, psS, AF.Exp, scale=SCALE)
                        etiles[i] = e

                    psO2 = pso.tile([128, CH2], F32, tag="o", name="psO2")
                    scores(0)
                    scores(1)
                    accs = None
                    for i in range(NT):
                        if i == 0 and tail_prev[0] is not None:
                            tail_prev[0]()
                        if i == 0:
                            # allocated after the previous tail so WAR
                            # deps resolve in emission order
                            accs = [
                                accpool.tile([128, CH2], BF16, tag=f"acc{_c}", name=f"acc{_c}")
                                for _c in range(4)
                            ]
                        e = etiles[i]
                        for h2 in range(2):
                            eh = e[:, ds(h2 * CH, CH)]
                            nc.tensor.matmul(
                                psO2[:, ds(h2 * CH, CH)],
                                VT[:, i, :],
                                eh,
                                start=(i == 0),
                                stop=(i == NT - 1),
                            )
                        if i + 2 < NT:
                            scores(i + 2)
                        # softmax denominator: 4 stride-4 partial-sum
                        # chains on DVE; short colsum matmuls happen in
                        # the deferred tail.
                        g = i % 4
                        eng = nc.vector
                        if 4 <= i < 8:
                            eng.tensor_add(accs[g], etiles[i - 4], e)
                        elif i >= 8:
                            eng.tensor_add(accs[g], accs[g], e)
                        # pump only after the deferred tail was emitted:
                        # pumped oproj units read QA slices the tail writes,
                        # and tile deps resolve in emission order
                        if i >= 1:
                            pump(2)
                    # evict psO2 so the single PSUM buffer frees after one
                    # DVE op instead of after the deferred tail
                    O_sb = osbp.tile([128, CH2], F32, tag="osb", name="O_sb")
                    nc.vector.tensor_copy(O_sb, psO2)
                    tail_prev[0] = make_tail(hl, cp, O_sb, accs)

                def pump_noop(n):
                    pass

                if p == 0 or "oproj" not in phases:
                    for jh in range(QHP):
                        for cp in range(S // CH2):
                            attn_cp(p * QHP + jh, cp, pump_noop)
                else:
                    # pass 1 cp-major: all heads' cp0 first, then cp1 with
                    # oproj token tiles 0..7 pumped in, once every head's
                    # cp0 tail has flushed
                    for jh in range(QHP):
                        attn_cp(QHP + jh, 0, pump_noop)
                    og = oproj_gen(range(0, NT // 2))
                    def pump(n):
                        for _ in range(n):
                            if next(og, None) is None:
                                return
                    for jh in range(QHP):
                        attn_cp(QHP + jh, 1, pump)
                if tail_prev[0] is not None:
                    tail_prev[0]()
                    tail_prev[0] = None

        # ---------------- output projection ----------------
        if "oproj" not in phases:
            return
        for _ in oproj_gen(range(NT // 2, NT)):
            pass


_PROGRAM = None


def _build_program(legalize=True, bodies=1, phases=("proj", "attn", "oproj")):
    global _PROGRAM
    if _PROGRAM is not None and legalize and bodies == 1 and len(phases) == 3:
        return _PROGRAM
    nc = bass.Bass("TRN2", target_bir_lowering=False, debug=False, num_devices=NCORES)
    xT = nc.dram_tensor("xT", [HID, S], BF16, kind="ExternalInput").ap()
    wq = nc.dram_tensor("wq", [HID, QH * D], BF16, kind="ExternalInput").ap()
    wkv = nc.dram_tensor("wkv", [HID, KVH, 256], BF16, kind="ExternalInput").ap()
    rope4 = nc.dram_tensor("rope4", [S, 4, D], F32, kind="ExternalInput").ap()
    wo = nc.dram_tensor("wo", [QH * D, HID], BF16, kind="ExternalInput").ap()
    ones_d = nc.dram_tensor("ones", [128, 128], F32R, kind="ExternalInput").ap()
    ones_bf = nc.dram_tensor("ones_bf", [128, 2], BF16, kind="ExternalInput").ap()
    out = nc.dram_tensor("out", [S, HID], F32, kind="ExternalOutput").ap()
    with tile.TileContext(nc) as tc:
        for _rep in range(bodies):
            _emit(nc, tc, (xT, wq, wkv, rope4, wo, ones_d, ones_bf, out), phases=phases)
    if legalize:
        _legalize_waits(nc)
        if bodies == 1 and len(phases) == 3:
            _PROGRAM = nc
    return nc


def _host_prep(hidden_states, cos, sin, Wq, Wk, Wv, Wo, q_norm_w, k_norm_w):
    """Build per-core input maps."""
    f = np.float32
    cos = np.asarray(cos, f)
    sin = np.asarray(sin, f)
    qw = np.asarray(q_norm_w, f)
    kw = np.asarray(k_norm_w, f)

    def fold(w):
        cw = (cos * w[None, :]).astype(f)
        sw = np.empty_like(sin)
        half = D // 2
        sw[:, :half] = -sin[:, :half] * w[None, half:]
        sw[:, half:] = sin[:, half:] * w[None, :half]
        return np.ascontiguousarray(cw), np.ascontiguousarray(sw)

    cwq, swq = fold(qw)
    cwk, swk = fold(kw)
    rope4 = np.stack([cwq, swq, cwk, swk], axis=1)  # [S, 4, D]

    Wq = np.asarray(Wq, f)
    Wk = np.asarray(Wk, f)
    Wv = np.asarray(Wv, f)
    Wo = np.asarray(Wo, f)
    hs = np.asarray(hidden_states, f)

    bf = ml_dtypes.bfloat16
    in_maps = []
    for i in range(NCORES):
        b, g = i // TP, i % TP
        xT = np.ascontiguousarray(hs[b].T).astype(bf)           # [HID, S]
        wq_g = np.ascontiguousarray(Wq[:, g * QH * D:(g + 1) * QH * D]).astype(bf)
        wkv = np.empty((HID, KVH, 256), f)
        for p in range(KVH):
            kvh = g * KVH + p
            wkv[:, p, 0:128] = Wk[:, kvh * D:(kvh + 1) * D]
            wkv[:, p, 128:256] = Wv[:, kvh * D:(kvh + 1) * D]
        wkv = wkv.astype(bf)
        wo_g = np.ascontiguousarray(Wo[g * QH * D:(g + 1) * QH * D, :]).astype(bf)
        in_maps.append(
            {
                "xT": xT,
                "wq": wq_g,
                "wkv": wkv,
                "rope4": rope4,
                "wo": wo_g,
                "ones": np.ones((128, 128), f),
                "ones_bf": np.ones((128, 2), ml_dtypes.bfloat16),
            }
        )
    return in_maps


def run_cores(in_maps, trace=False, **kwargs):
    nc = _build_program()
    return run_bass_kernel_spmd(
        nc, in_maps, core_ids=list(range(NCORES)), trace=trace, **kwargs
    )


def kernel(hidden_states, cos, sin, Wq, Wk, Wv, Wo, q_norm_w, k_norm_w):
    in_maps = _host_prep(
        hidden_states, cos, sin, Wq, Wk, Wv, Wo, q_norm_w, k_norm_w
    )
    res = run_cores(in_maps, trace=False)
    out = np.empty((B, S, HID), np.float32)
    for b in range(B):
        out[b] = res.results[b * TP]["out"]
        out[b] += res.results[b * TP + 1]["out"]
    return out



# revision 9
# speedup vs baseline: 1.0281x; 1.0065x over previous
"""AceStep GQA attention block on 8 TRN2 NeuronCores.

Sharding: tensor-parallel over heads (TP=2, kv heads stay grouped with
their q heads) x data-parallel over batch (DP=4).  Core i handles batch
b = i // 2 and head group g = i % 2 (q heads 8g..8g+7, kv heads 2g,2g+1).
Each core computes a partial output projection (its head group's slice of
Wo rows); the host sums the two partials per batch.

Device-side dataflow per core (all matmuls f32r = full-rate fp32):
  pass p in {0,1}:  (kv head p, q heads 4p..4p+3)
    proj:  xT tiles (stationary) x Wq/Wk/Wv slices -> Q/K/V token-major,
           per-head RMSNorm via ACT square+accum, rsqrt; RoPE fused with the
           norm scale via scalar_tensor_tensor (cos/sin tables carry the
           norm weights, folded on host); PE-transpose Q,K to [d, t].
    attn:  ST[sk,sq] = KT_tile.T @ QT chunk; E = exp(SCALE*ST) on ACT;
           denominator = ones.T @ E (PE, accumulated over sk tiles);
           OUT_T[d,sq] = V_tile.T @ E accumulated over sk tiles;
           A = OUT_T * bcast(1/denom)  (bcast via K=1 matmul).
  final: out[t,:] += A_h[:,t].T @ Wo_h rows, accumulated over 8 heads.
"""

import sys

if "/opt/trn_rl_repo" not in sys.path:
    sys.path.insert(0, "/opt/trn_rl_repo")

from contextlib import ExitStack

import numpy as np
import ml_dtypes

import concourse.bass as bass
import concourse.mybir as mybir
import concourse.tile as tile
from concourse.bass import ts, ds
from concourse.masks import make_identity
from concourse.vector_clock import ScopedClock, VectorClock
from concourse.bass_utils import run_bass_kernel_spmd

B, S, HID = 4, 2048, 2048
H, KV, D = 16, 4, 128
EPS = 1e-6
SCALE = float(D) ** -0.5
NCORES = 8
TP = 2
QH = H // TP            # 8 q heads per core
KVH = KV // TP          # 2 kv heads per core = passes
QHP = QH // KVH         # 4 q heads per pass
NT = S // 128           # 16 token tiles
NHID = HID // 128       # 16 hid tiles
CH = 512                # sq chunk width
NCH = S // CH           # 4 chunks
F32 = mybir.dt.float32
F32R = mybir.dt.float32r
BF16 = mybir.dt.bfloat16
CH2 = 1024              # paired sq chunk width (2 PSUM banks)
MULT = mybir.AluOpType.mult
AF = mybir.ActivationFunctionType


def _patched_drain_and_barrier(self, tick_clock, wait_clock):
    # Walrus CoreV3 rejects >1-2 sem waits on a CTRL (Drain) instruction.
    # Split the final global-clock wait into one single-wait drain per proc.
    gc = tick_clock.global_clock
    n = len(gc)
    for p in range(n):
        t = gc[p]
        if t > 0:
            vec = [0] * n
            vec[p] = t
            d = self.nc.sync.drain()
            wait_clock.add_sem_waits(d.ins, ScopedClock({None: VectorClock(vec)}))
    self.nc.sync.drain()
    self.nc.all_engine_barrier()
    assert self.sems is not None
    popped = self.nc._tile_sem_poison_stack.pop()
    assert popped is self._sem_poison
    self.nc.clear_and_free_semaphores(list(self.sems.allocated().values()))
    self.nc.all_engine_barrier()


tile.TileContext._drain_and_barrier = _patched_drain_and_barrier

def _max_waits(inst):
    # Walrus CoreV2/V3 setupSyncWait takes a single wait per TPB instruction;
    # EventSemaphore can hold two.
    if isinstance(inst, mybir.InstEventSemaphore):
        return 2
    return 1


def _legalize_waits(nc):
    """Walrus CoreV3 rejects instructions carrying too many sync waits.
    Spill the excess onto no-op carrier instructions inserted just before,
    on the same engine stream."""
    n_new = 0
    for f in nc.m.functions:
        for bb in f.blocks:
            insts = bb.instructions
            out = []
            changed = False
            for inst in insts:
                si = getattr(inst, "sync_info", None)
                waits = list(si.on_wait) if (si and si.on_wait) else []
                mw = _max_waits(inst)
                if len(waits) > mw:
                    spill, keep = waits[:-mw], waits[-mw:]
                    for i in range(0, len(spill)):
                        nop = mybir.InstNoOp(
                            name=f"waitspill-{n_new}",
                            engine=inst.engine,
                            sync_info=mybir.SyncInfo(
                                on_wait=spill[i : i + 1], on_update=[]
                            ),
                            bass_nofuse=True,
                        )
                        n_new += 1
                        out.append(nop)
                    si.on_wait = keep
                    changed = True
                out.append(inst)
            if changed:
                bb.instructions = out
    return n_new


def _emit(nc, tc, io, phases=("proj", "attn", "oproj")):
    xT, wq, wkv, rope4, wo, ones_d, ones_bf, out = io

    xT = xT.rearrange("(j p) t -> p j t", p=128)       # [128, NHID, S]
    wq = wq.rearrange("(j p) n -> p j n", p=128)       # [128, NHID, QH*D]
    wkv = wkv.rearrange("(j p) a n -> p j a n", p=128)  # [128, NHID, KVH, 256]
    wo_r = wo.rearrange("(h p) n -> p h n", p=128)     # [128, QH, HID]

    with ExitStack() as top:
        const = top.enter_context(tc.tile_pool(name="const", bufs=1))
        ident = const.tile([128, 128], F32)
        make_identity(nc, ident)
        ones_col = const.tile([128, 1], BF16)
        nc.sync.dma_start(out=ones_col, in_=ones_bf[:, 0:1])
        ones_row = const.tile([1, 128], F32R)
        nc.sync.dma_start(out=ones_row, in_=ones_d[0:1, :])
        eps_t = const.tile([128, 1], F32)
        nc.vector.memset(eps_t, EPS)
        # pin the ACT table set to natural_log_exp_and_others (has exp, ln,
        # square, copy) so no table switches happen mid-kernel
        dummy = const.tile([128, 1], F32)
        nc.scalar.activation(dummy, eps_t, AF.Ln)

        qa_pool = top.enter_context(tc.tile_pool(name="qa", bufs=1))
        QA = qa_pool.tile([128, QH, S], BF16)           # QT, later A, [d, h, t]
        kt_pool = top.enter_context(tc.tile_pool(name="kt", bufs=1))
        KT = kt_pool.tile([128, S], BF16)               # per-pass KT [d, t]
        v_pool = top.enter_context(tc.tile_pool(name="v", bufs=1))
        VT = v_pool.tile([128, NT, D], BF16)            # per-pass V [t%128, tt, d]

        epool = top.enter_context(tc.tile_pool(name="e", bufs=10))
        wq_pool = top.enter_context(tc.tile_pool(name="wq", bufs=2))

        def load_weights(p):
            wq_sb = wq_pool.tile([128, NHID, QHP * D], BF16, tag="wq", name="wq_sb")
            wkv_sb = wq_pool.tile([128, NHID, 256], BF16, tag="wkv", name="wkv_sb")
            for jq in range(4):
                nc.scalar.dma_start(
                    out=wq_sb[:, ds(jq * 4, 4), :],
                    in_=wq[:, ds(jq * 4, 4), ds(p * QHP * D, QHP * D)],
                )
            nc.scalar.dma_start(out=wkv_sb, in_=wkv[:, :, p, :])
            return wq_sb, wkv_sb

        cur_w = load_weights(0) if "proj" in phases else None
        small = top.enter_context(tc.tile_pool(name="small", bufs=4))

        wo_sb = None
        oproj_pools = [None, None]  # (psc, opool)

        def oproj_gen(tts):
            psc_p, opool_p = oproj_pools
            for tt in tts:
                for half in range(2):
                    osb = opool_p.tile([128, 2, CH], F32, tag="osb", name="osb")
                    for nc2 in range(2):
                        nch = half * 2 + nc2
                        psC = psc_p.tile([128, CH], F32, tag="c", name="psC")
                        for h in range(QH):
                            nc.tensor.matmul(
                                psC,
                                QA[:, h, ts(tt, 128)],
                                wo_sb[:, h, ds(nch * CH, CH)],
                                start=(h == 0),
                                stop=(h == QH - 1),
                            )
                            if h % 4 == 3:
                                yield True
                        if nch % 2 == 0:
                            nc.scalar.copy(osb[:, nc2, :], psC)
                        else:
                            nc.vector.tensor_copy(osb[:, nc2, :], psC)
                        yield True
                    nc.sync.dma_start(
                        out=out[ts(tt, 128), ds(half * 2 * CH, 2 * CH)].rearrange(
                            "p (a c) -> p a c", a=2
                        ),
                        in_=osb,
                    )
                    yield True

        for p in range(KVH):
            # ---------------- projection phase (pass p) ----------------
            with ExitStack() as ph:
                if "proj" in phases:
                    wq_sb, wkv_sb = cur_w
                    xpool = ph.enter_context(tc.tile_pool(name="x", bufs=6))
                    rpool = ph.enter_context(tc.tile_pool(name="rope", bufs=3))
                    spool = ph.enter_context(tc.tile_pool(name="scr", bufs=4))
                    qrpool = ph.enter_context(tc.tile_pool(name="qr", bufs=10))
                    psq = ph.enter_context(tc.tile_pool(name="psq", bufs=3, space="PSUM"))
                    pskv = ph.enter_context(tc.tile_pool(name="pskv", bufs=2, space="PSUM"))
                    pst_pool = ph.enter_context(
                        tc.tile_pool(name="pst", bufs=3, space="PSUM")
                    )

                    # transpose+copy of tile tt is deferred until after tile
                    # tt+1's projection matmuls so the PE never waits on the
                    # ACT/DVE norm+rope chain.
                    pending = []
                    new_pending = []

                    def flush_pending():
                        for qr_t, dst in pending:
                            psT = pst_pool.tile([128, 128], F32)
                            nc.tensor.transpose(psT, qr_t, ident)
                            nc.scalar.copy(dst, psT)
                        pending.clear()

                    for tt in range(NT):
                        xx = xpool.tile([128, NHID, 128], BF16, tag="xx")
                        nc.sync.dma_start(out=xx, in_=xT[:, :, ts(tt, 128)])
                        rp = rpool.tile([128, 4, 128], F32, tag="rp")
                        nc.sync.dma_start(out=rp, in_=rope4[ts(tt, 128), :, :])
                        cwq_t = rp[:, 0, :]
                        swq_t = rp[:, 1, :]
                        cwk_t = rp[:, 2, :]
                        swk_t = rp[:, 3, :]

                        psQ = psq.tile([128, QHP * D], F32)
                        psKV = pskv.tile([128, 256], F32)
                        for j in range(NHID):
                            nc.tensor.matmul(
                                psQ,
                                xx[:, j, :],
                                wq_sb[:, j, :],
                                start=(j == 0),
                                stop=(j == NHID - 1),
                            )
                        for j in range(NHID):
                            nc.tensor.matmul(
                                psKV,
                                xx[:, j, :],
                                wkv_sb[:, j, :],
                                start=(j == 0),
                                stop=(j == NHID - 1),
                            )

                        # batched RMSNorm scale: 5 squares (4 Q heads + K)
                        # accumulate into one [128,5]; one ln + one exp.
                        scratch = spool.tile([128, 128], F32, tag="scr")
                        ssq5 = small.tile([128, 5], F32, tag="ssq")
                        s15 = small.tile([128, 5], F32, tag="s1")
                        r5 = small.tile([128, 5], F32, tag="r")
                        for jh in range(QHP):
                            nc.scalar.activation(
                                scratch,
                                psQ[:, ts(jh, 128)],
                                AF.Square,
                                accum_out=ssq5[:, jh : jh + 1],
                            )
                        nc.scalar.activation(
                            scratch,
                            psKV[:, 0:128],
                            AF.Square,
                            accum_out=ssq5[:, 4:5],
                        )
                        nc.scalar.activation(s15, ssq5, AF.Ln, bias=eps_t, scale=1.0 / D)
                        nc.scalar.activation(r5, s15, AF.Exp, scale=-0.5)

                        def norm_rope(src, cw_t, sw_t, r, dst):
                            m1 = spool.tile([128, 128], F32, tag="m1")
                            m2 = spool.tile([128, 128], F32, tag="m2")
                            qr = qrpool.tile([128, 128], F32, tag="qr")
                            nc.vector.scalar_tensor_tensor(
                                out=m1, in0=src, scalar=r, in1=cw_t, op0=MULT, op1=MULT
                            )
                            nc.vector.scalar_tensor_tensor(
                                out=m2[:, 0:64],
                                in0=src[:, 64:128],
                                scalar=r,
                                in1=sw_t[:, 0:64],
                                op0=MULT,
                                op1=MULT,
                            )
                            nc.vector.scalar_tensor_tensor(
                                out=m2[:, 64:128],
                                in0=src[:, 0:64],
                                scalar=r,
                                in1=sw_t[:, 64:128],
                                op0=MULT,
                                op1=MULT,
                            )
                            nc.vector.tensor_add(qr, m1, m2)
                            new_pending.append((qr, dst))

                        for jh in range(QHP):
                            hl = p * QHP + jh
                            norm_rope(
                                psQ[:, ts(jh, 128)],
                                cwq_t,
                                swq_t,
                                r5[:, jh : jh + 1],
                                QA[:, hl, ts(tt, 128)],
                            )
                        norm_rope(
                            psKV[:, 0:128],
                            cwk_t,
                            swk_t,
                            r5[:, 4:5],
                            KT[:, ts(tt, 128)],
                        )
                        nc.scalar.copy(VT[:, tt, :], psKV[:, 128:256])
                        flush_pending()
                        pending.extend(new_pending)
                        new_pending.clear()
                    flush_pending()
                    if p + 1 < KVH:
                        cur_w = load_weights(p + 1)

            # load Wo after the last projection phase frees its pools
            if p == KVH - 1 and "oproj" in phases:
                wo_pool = top.enter_context(tc.tile_pool(name="wo", bufs=1))
                wo_sb = wo_pool.tile([128, QH, HID], BF16)
                nc.sync.dma_start(out=wo_sb, in_=wo_r)
                oproj_pools[0] = top.enter_context(
                    tc.tile_pool(name="psc", bufs=2, space="PSUM")
                )
                oproj_pools[1] = top.enter_context(
                    tc.tile_pool(name="osb2", bufs=3)
                )

            # ---------------- attention phase (pass p) ----------------
            if "attn" not in phases:
                continue
            # Processed in sq chunk-pairs of 1024: scores fill a 2-bank PSUM
            # tile, one wide exp per sk tile (amortizes ACT per-op overhead),
            # denominator 2-way column-tiled on the PE (concurrent groups).
            with ExitStack() as ph:
                accpool = ph.enter_context(tc.tile_pool(name="acc", bufs=1))
                osbp = ph.enter_context(tc.tile_pool(name="osbp", bufs=2))
                pss = ph.enter_context(tc.tile_pool(name="pss", bufs=2, space="PSUM"))
                pso = ph.enter_context(tc.tile_pool(name="pso", bufs=1, space="PSUM"))

                # The normalization tail of chunk-pair K (denominator
                # colsum, reciprocal, broadcast, final multiply) is deferred
                # into chunk-pair K+1's stream so the PE never waits on the
                # DVE/Pool partial-sum chains.
                tail_prev = [None]

                def make_tail(hl, cp, O_sb, accs):
                    def tail():
                        psD = [
                            pss.tile([1, CH], F32, tag="s", name=f"psD{_h}")
                            for _h in range(2)
                        ]
                        for h2 in range(2):
                            for gi in range(4):
                                nc.tensor.matmul(
                                    psD[h2],
                                    ones_col,
                                    accs[gi][:, ds(h2 * CH, CH)],
                                    start=(gi == 0),
                                    stop=(gi == 3),
                                )
                        for h2 in range(2):
                            c = cp * 2 + h2
                            rd = small.tile([1, CH], F32R, tag="rd")
                            with nc.allow_low_precision(reason="f32r bcast rhs"):
                                nc.vector.reciprocal(rd, psD[h2])
                            psB = pss.tile([128, CH], F32, tag="s")
                            nc.tensor.matmul(
                                psB, ones_row, rd, start=True, stop=True
                            )
                            bc = epool.tile([128, CH], F32, tag="bc")
                            nc.vector.tensor_copy(bc, psB)
                            nc.vector.tensor_mul(
                                QA[:, hl, ds(c * CH, CH)],
                                O_sb[:, ds(h2 * CH, CH)],
                                bc,
                            )
                    return tail

                def attn_cp(hl, cp, pump):
                    etiles = [None] * NT

                    def scores(i):
                        psS = pss.tile([128, CH2], F32, tag="s")
                        for h2 in range(2):
                            nc.tensor.matmul(
                                psS[:, ds(h2 * CH, CH)],
                                KT[:, ts(i, 128)],
                                QA[:, hl, ds(cp * CH2 + h2 * CH, CH)],
                                start=True,
                                stop=True,
                            )
                        e = epool.tile([128, CH2], BF16, tag="e")
                        nc.scalar.activation(e, psS, AF.Exp, scale=SCALE)
                        etiles[i] = e

                    psO2 = pso.tile([128, CH2], F32, tag="o", name="psO2")
                    scores(0)
                    scores(1)
                    accs = None
                    for i in range(NT):
                        if i == 0 and tail_prev[0] is not None:
                            tail_prev[0]()
                        if i == 0:
                            # allocated after the previous tail so WAR
                            # deps resolve in emission order
                            accs = [
                                accpool.tile([128, CH2], BF16, tag=f"acc{_c}", name=f"acc{_c}")
                                for _c in range(4)
                            ]
                        e = etiles[i]
                        for h2 in range(2):
                            eh = e[:, ds(h2 * CH, CH)]
                            nc.tensor.matmul(
                                psO2[:, ds(h2 * CH, CH)],
                                VT[:, i, :],
                                eh,
                                start=(i == 0),
                                stop=(i == NT - 1),
                            )
                        if i + 2 < NT:
                            scores(i + 2)
                        # softmax denominator: 4 stride-4 partial-sum
                        # chains on DVE; short colsum matmuls happen in
                        # the deferred tail.
                        g = i % 4
                        eng = nc.vector
                        if 4 <= i < 8:
                            eng.tensor_add(accs[g], etiles[i - 4], e)
                        elif i >= 8:
                            eng.tensor_add(accs[g], accs[g], e)
                        # pump only after the deferred tail was emitted:
                        # pumped oproj units read QA slices the tail writes,
                        # and tile deps resolve in emission order
                        if i >= 1:
                            pump(2)
                    # evict psO2 so the single PSUM buffer frees after one
                    # DVE op instead of after the deferred tail
                    O_sb = osbp.tile([128, CH2], F32, tag="osb", name="O_sb")
                    nc.vector.tensor_copy(O_sb, psO2)
                    tail_prev[0] = make_tail(hl, cp, O_sb, accs)

                def pump_noop(n):
                    pass

                if p == 0 or "oproj" not in phases:
                    for jh in range(QHP):
                        for cp in range(S // CH2):
                            attn_cp(p * QHP + jh, cp, pump_noop)
                else:
                    # pass 1 cp-major: all heads' cp0 first, then cp1 with
                    # oproj token tiles 0..7 pumped in, once every head's
                    # cp0 tail has flushed
                    for jh in range(QHP):
                        attn_cp(QHP + jh, 0, pump_noop)
                    og = oproj_gen(range(0, NT // 2))
                    def pump(n):
                        for _ in range(n):
                            if next(og, None) is None:
                                return
                    for jh in range(QHP):
                        attn_cp(QHP + jh, 1, pump)
                if tail_prev[0] is not None:
                    tail_prev[0]()
                    tail_prev[0] = None

        # ---------------- output projection ----------------
        if "oproj" not in phases:
            return
        for _ in oproj_gen(range(NT // 2, NT)):
            pass


_PROGRAM = None


def _build_program(legalize=True, bodies=1, phases=("proj", "attn", "oproj")):
    global _PROGRAM
    if _PROGRAM is not None and legalize and bodies == 1 and len(phases) == 3:
        return _PROGRAM
    nc = bass.Bass("TRN2", target_bir_lowering=False, debug=False, num_devices=NCORES)
    xT = nc.dram_tensor("xT", [HID, S], BF16, kind="ExternalInput").ap()
    wq = nc.dram_tensor("wq", [HID, QH * D], BF16, kind="ExternalInput").ap()
    wkv = nc.dram_tensor("wkv", [HID, KVH, 256], BF16, kind="ExternalInput").ap()
    rope4 = nc.dram_tensor("rope4", [S, 4, D], F32, kind="ExternalInput").ap()
    wo = nc.dram_tensor("wo", [QH * D, HID], BF16, kind="ExternalInput").ap()
    ones_d = nc.dram_tensor("ones", [128, 128], F32R, kind="ExternalInput").ap()
    ones_bf = nc.dram_tensor("ones_bf", [128, 2], BF16, kind="ExternalInput").ap()
    out = nc.dram_tensor("out", [S, HID], F32, kind="ExternalOutput").ap()
    with tile.TileContext(nc) as tc:
        for _rep in range(bodies):
            _emit(nc, tc, (xT, wq, wkv, rope4, wo, ones_d, ones_bf, out), phases=phases)
    if legalize:
        _legalize_waits(nc)
        if bodies == 1 and len(phases) == 3:
            _PROGRAM = nc
    return nc


def _host_prep(hidden_states, cos, sin, Wq, Wk, Wv, Wo, q_norm_w, k_norm_w):
    """Build per-core input maps."""
    f = np.float32
    cos = np.asarray(cos, f)
    sin = np.asarray(sin, f)
    qw = np.asarray(q_norm_w, f)
    kw = np.asarray(k_norm_w, f)

    def fold(w):
        cw = (cos * w[None, :]).astype(f)
        sw = np.empty_like(sin)
        half = D // 2
        sw[:, :half] = -sin[:, :half] * w[None, half:]
        sw[:, half:] = sin[:, half:] * w[None, :half]
        return np.ascontiguousarray(cw), np.ascontiguousarray(sw)

    cwq, swq = fold(qw)
    cwk, swk = fold(kw)
    rope4 = np.stack([cwq, swq, cwk, swk], axis=1)  # [S, 4, D]

    Wq = np.asarray(Wq, f)
    Wk = np.asarray(Wk, f)
    Wv = np.asarray(Wv, f)
    Wo = np.asarray(Wo, f)
    hs = np.asarray(hidden_states, f)

    bf = ml_dtypes.bfloat16
    in_maps = []
    for i in range(NCORES):
        b, g = i // TP, i % TP
        xT = np.ascontiguousarray(hs[b].T).astype(bf)           # [HID, S]
        wq_g = np.ascontiguousarray(Wq[:, g * QH * D:(g + 1) * QH * D]).astype(bf)
        wkv = np.empty((HID, KVH, 256), f)
        for p in range(KVH):
            kvh = g * KVH + p
            wkv[:, p, 0:128] = Wk[:, kvh * D:(kvh + 1) * D]
            wkv[:, p, 128:256] = Wv[:, kvh * D:(kvh + 1) * D]
        wkv = wkv.astype(bf)
        wo_g = np.ascontiguousarray(Wo[g * QH * D:(g + 1) * QH * D, :]).astype(bf)
        in_maps.append(
            {
                "xT": xT,
                "wq": wq_g,
                "wkv": wkv,
                "rope4": rope4,
                "wo": wo_g,
                "ones": np.ones((128, 128), f),
                "ones_bf": np.ones((128, 2), ml_dtypes.bfloat16),
            }
        )
    return in_maps


def run_cores(in_maps, trace=False, **kwargs):
    nc = _build_program()
    return run_bass_kernel_spmd(
        nc, in_maps, core_ids=list(range(NCORES)), trace=trace, **kwargs
    )


def kernel(hidden_states, cos, sin, Wq, Wk, Wv, Wo, q_norm_w, k_norm_w):
    in_maps = _host_prep(
        hidden_states, cos, sin, Wq, Wk, Wv, Wo, q_norm_w, k_norm_w
    )
    res = run_cores(in_maps, trace=False)
    out = np.empty((B, S, HID), np.float32)
    for b in range(B):
        out[b] = res.results[b * TP]["out"]
        out[b] += res.results[b * TP + 1]["out"]
    return out

